# revision 1
# baseline (speedup 1.0000x reference)
"""CapsNet forward on 8 TRN2 NeuronCores — data-parallel over batch.

Device (per core, batch shard of 32): conv1 (9x9 s1 + relu) and the primary-caps
conv (9x9 s2) as fp16 matmuls (fp32 PSUM accumulate) against an SBUF-resident
feature map; conv1 is fed host-side im2col patches.  Host: squash + capsule
transform + 3 routing iterations (batch-global, tiny FLOP count) in numpy.

Layouts (per core):
  p1  [256, 12800] fp16   im2col patches, rows = (c,ky,kx) K-index (243 pad 256),
                          cols = (y, x, b) y-major, batch innermost
  w1  [256, 256]   fp16   rows = K, cols = out-channel (c2 caps-major)
  w2  [81*256,256] fp16   rows = (ky,kx,cin), cols = out-channel
  u   [256, 1152]  f32    rows = out-channel, cols per MODE (_unshard_u inverts)

Perf notes (cost-model timeline, per core): PE-bound ~183 us busy / ~201 us
total. fp16 matmuls (fp32 PSUM) run 4x faster than fp32 on the PE. conv2 uses
the minimum matmul count (3 x 384-col MMs per K-tile; PSUM bank = 512 fp32
caps a single MM's N). w2 prefetch is ordered after the p1 stream because
conv1 alone needs the full ~358 GB/s HBM bandwidth to keep the PE fed.
"""

import numpy as np

NUM_PRIMARY = 8
NUM_SHAPE = 10
NUM_ROUTES = 32 * 6 * 6  # 1152
B = 256
NCORES = 8
BC = B // NCORES  # 32
P = 128

FN = BC * 400  # 12800 conv1 output cols per core, (y20, x20, b32)
CH = 1024      # conv1 chunk cols (psum tile = 2 banks; 13th chunk is 512)


MODE = "n384"  # conv2 structure: win288 | flat288 | n384 | im2col


def _build_program(use_dve=True, reps=1, mode=None):
    mode = mode or MODE
    import concourse.mybir as mybir
    import concourse.tile as tile
    from concourse import bacc
    from contextlib import ExitStack

    f32 = mybir.dt.float32
    f16 = mybir.dt.float16
    nc = bacc.Bacc("TRN2", target_bir_lowering=False, debug=False,
                   num_devices=NCORES)
    p1 = nc.dram_tensor("p1", [256, FN], f16, kind="ExternalInput").ap()
    w1 = nc.dram_tensor("w1", [256, 256], f16, kind="ExternalInput").ap()
    w2 = nc.dram_tensor("w2", [81 * 256, 256], f16, kind="ExternalInput").ap()
    b1d = nc.dram_tensor("b1", [256, 1], f32, kind="ExternalInput").ap()
    pbd = nc.dram_tensor("pb", [256, 1], f32, kind="ExternalInput").ap()
    uo_dt = f16 if mode == "n384" else f32
    uo = nc.dram_tensor("u_out", [256, BC * 36], uo_dt,
                        kind="ExternalOutput").ap()

    with tile.TileContext(nc) as tc, ExitStack() as ctx:
        const = ctx.enter_context(tc.tile_pool(name="const", bufs=1))
        w1_sb = const.tile([P, 2, 256], f16)
        nc.sync.dma_start(w1_sb[:], w1.rearrange("(t p) m -> p t m", p=P))
        # biases go on the ACT HWDGE ring so their DMA setups don't delay
        # the first p1 chunk on the SP ring; they are also ordered after
        # chunk0 below (not needed until the first evacuation)
        b1_sb = const.tile([P, 2], f32)
        bias_dmas = [
            nc.scalar.dma_start(b1_sb[:],
                                b1d.rearrange("(t p) o -> p (t o)", p=P))]
        pb_sb = const.tile([P, 2], f32)
        bias_dmas.append(
            nc.scalar.dma_start(pb_sb[:],
                                pbd.rearrange("(t p) o -> p (t o)", p=P)))

        # conv2 weight tiles: 9 (one per ky); DMAs are issued AFTER the conv1
        # chunk loop so p1 gets full HBM bandwidth while conv1 runs (w2 is
        # only consumed at ~70 GB/s once conv2 starts)
        w2v = w2.rearrange("(k t p) m -> k p t m", p=P, t=2)
        w2pool = ctx.enter_context(tc.tile_pool(name="w2", bufs=1))
        w2_sb = [w2pool.tile([P, 2, 256], f16, tag=f"w2k_{k}", name=f"w2k_{k}")
                 for k in range(81)]

        hpool = ctx.enter_context(tc.tile_pool(name="h", bufs=1))
        h_sb = [hpool.tile([P, FN], f16, tag=f"h{t}", name=f"h{t}") for t in range(2)]
        upool = ctx.enter_context(tc.tile_pool(name="u", bufs=1))
        u_sb = [upool.tile([P, BC * 36], f32, tag=f"u{t}", name=f"u{t}")
                for t in range(2)]

        # ---- conv1: h[m, pos] = relu(w1[:,m]^T @ p1[:,pos] + b1[m]) ----
        p1v = p1.rearrange("(t p) n -> p t n", p=P)
        p1_dmas = []
        for rep in range(reps):
          with tc.tile_pool(name=f"p1pool{rep}", bufs=3) as p1pool, \
             tc.tile_pool(name=f"psum1_{rep}", bufs=2, space="PSUM") as psum1:
            for off in range(0, FN, CH):
                cw = min(CH, FN - off)
                pt = p1pool.tile([P, 2, CH], f16)
                d = nc.sync.dma_start(pt[:, :, :cw], p1v[:, :, off:off + cw])
                if rep == 0:
                    if not p1_dmas:
                        from concourse.tile import add_dep_helper
                        for bd in bias_dmas:
                            add_dep_helper(bd.ins, d.ins,
                                           reason="bias DMA after chunk0")
                    p1_dmas.append(d)
                for oct in range(2):
                    ps = psum1.tile([P, CH], f32, tag=f"ps{oct}", name=f"ps{oct}")
                    for n0 in range(0, cw, 512):
                        nn = min(512, cw - n0)
                        for t in range(2):
                            nc.tensor.matmul(
                                ps[:, n0:n0 + nn],
                                w1_sb[:, t, oct * P:(oct + 1) * P],
                                pt[:, t, n0:n0 + nn],
                                start=(t == 0), stop=(t == 1))
                    if oct == 0 or not use_dve:
                        nc.scalar.activation(
                            h_sb[oct][:, off:off + cw], ps[:, :cw],
                            mybir.ActivationFunctionType.Relu,
                            bias=b1_sb[:, oct:oct + 1])
                    else:
                        nc.vector.tensor_scalar(
                            h_sb[1][:, off:off + cw], ps[:, :cw],
                            b1_sb[:, 1:2], 0.0,
                            mybir.AluOpType.add, mybir.AluOpType.max)

          if rep == 0:
              from concourse.tile import add_dep_helper
              for k81 in range(81):
                  # ACT ring: don't serialize setups behind p1 chunks;
                  # per-(ky,kx) granularity so conv2's first k-tile can
                  # start as soon as 131 KB (not 1.18 MB) has landed
                  d = nc.scalar.dma_start(w2_sb[k81][:], w2v[k81])
                  # keep the w2 stream off the HBM bus while conv1's input
                  # stream needs the full bandwidth (conv1 is DMA-bound,
                  # conv2 isn't); release it near the end of the p1 stream
                  add_dep_helper(d.ins, p1_dmas[-2].ins,
                                 reason="w2 prefetch after p1 stream")

          # ---- conv2: u[m, ...] = sum_k w2[k,:,m]^T @ h[:, win(k)] + pb ----
          hv = [h_sb[t][:].rearrange("p (y x b) -> p y x b", y=20, x=20)
                for t in range(2)]
          if mode in ("win288", "flat288"):
            # output col groups = 4 batch-groups of 8, cols (y6, x6, b8)
            with tc.tile_pool(name=f"psum2_{rep}", bufs=1, space="PSUM") as psum2:
              pg = [[psum2.tile([P, 288], f32, tag=f"pg{o}_{c}",
                                name=f"pg{rep}_{o}_{c}")
                     for c in range(4)] for o in range(2)]
              for ky in range(9):
                for kx in range(9):
                    k = ky * 9 + kx
                    for t in range(2):
                        for oct in range(2):
                            lhsT = w2_sb[k][:, t, oct * P:(oct + 1) * P]
                            for c in range(4):
                                if mode == "flat288":  # perf expt: wrong math
                                    rhs = h_sb[t][:, c * 288:(c + 1) * 288]
                                else:
                                    rhs = hv[t][:, ky:ky + 12:2, kx:kx + 12:2,
                                                c * 8:(c + 1) * 8]
                                nc.tensor.matmul(
                                    pg[oct][c][:], lhsT, rhs,
                                    start=(k == 0 and t == 0),
                                    stop=(k == 80 and t == 1))
              for oct in range(2):
                for c in range(4):
                    if oct == 0 or not use_dve:
                        nc.scalar.activation(
                            u_sb[oct][:, c * 288:(c + 1) * 288], pg[oct][c][:],
                            mybir.ActivationFunctionType.Identity,
                            bias=pb_sb[:, oct:oct + 1])
                    else:
                        nc.vector.tensor_scalar(
                            u_sb[1][:, c * 288:(c + 1) * 288], pg[1][c][:],
                            pb_sb[:, 1:2], None,
                            mybir.AluOpType.add, mybir.AluOpType.bypass)
          elif mode == "n384":
            # output col groups = 3 x-pairs, cols (y6, x2, b32): innermost
            # contiguous run is 32 fp16 elems (64 B)
            with tc.tile_pool(name=f"psum2_{rep}", bufs=1, space="PSUM") as psum2:
              pg = [[psum2.tile([P, 384], f32, tag=f"pg{o}_{g}",
                                name=f"pg{rep}_{o}_{g}")
                     for g in range(3)] for o in range(2)]
              for ky in range(9):
                for kx in range(9):
                    k = ky * 9 + kx
                    if k < 80:
                        order = [(t, oct, g) for t in range(2)
                                 for oct in range(2) for g in range(3)]
                    else:
                        # final k-tile: (oct,g)-major so each accumulator's
                        # last write retires early and its evacuation +
                        # store overlap the remaining matmuls
                        order = [(t, oct, g) for oct in range(2)
                                 for g in range(3) for t in range(2)]
                    for t, oct, g in order:
                        lhsT = w2_sb[k][:, t, oct * P:(oct + 1) * P]
                        rhs = hv[t][:, ky:ky + 12:2,
                                    kx + 4 * g:kx + 4 * g + 4:2, :]
                        nc.tensor.matmul(
                            pg[oct][g][:], lhsT, rhs,
                            start=(k == 0 and t == 0),
                            stop=(k == 80 and t == 1))
              uov_ = uo.rearrange("(t p) n -> t p n", p=P)
              for oct in range(2):
                ug = upool.tile([P, 1152], f16, tag=f"ug{oct}",
                                name=f"ug{rep}_{oct}")
                for g in range(3):
                    if oct == 0 or not use_dve:
                        nc.scalar.activation(
                            ug[:, g * 384:(g + 1) * 384], pg[oct][g][:],
                            mybir.ActivationFunctionType.Identity,
                            bias=pb_sb[:, oct:oct + 1])
                    else:
                        nc.vector.tensor_scalar(
                            ug[:, g * 384:(g + 1) * 384], pg[1][g][:],
                            pb_sb[:, 1:2], None,
                            mybir.AluOpType.add, mybir.AluOpType.bypass)
                nc.sync.dma_start(uov_[oct], ug[:])
          else:
            assert mode == "im2col"
            # materialize each window contiguously (ACT for t=0, DVE for t=1),
            # then fully-contiguous matmuls, cols (y6, x6, b32)
            with tc.tile_pool(name=f"psum2_{rep}", bufs=1, space="PSUM") as psum2, \
                 tc.tile_pool(name=f"vpool{rep}", bufs=3) as vpool:
              pg = [psum2.tile([P, 1152], f32, tag=f"pg{o}",
                               name=f"pg{rep}_{o}") for o in range(2)]
              for ky in range(9):
                for kx in range(9):
                    k = ky * 9 + kx
                    vt = [vpool.tile([P, 1152], f16, tag=f"v{t}",
                                     name=f"v{rep}_{k}_{t}") for t in range(2)]
                    for t in range(2):
                        src = hv[t][:, ky:ky + 12:2, kx:kx + 12:2, :]
                        dst = vt[t][:].rearrange("p (y x b) -> p y x b", y=6, x=6)
                        if t == 0 or not use_dve:
                            nc.scalar.copy(dst, src)
                        else:
                            nc.vector.tensor_copy(dst, src)
                    for t in range(2):
                        for oct in range(2):
                            lhsT = w2_sb[k][:, t, oct * P:(oct + 1) * P]
                            for n0 in (0, 512, 1024):
                                nn = min(512, 1152 - n0)
                                nc.tensor.matmul(
                                    pg[oct][:, n0:n0 + nn], lhsT,
                                    vt[t][:, n0:n0 + nn],
                                    start=(k == 0 and t == 0),
                                    stop=(k == 80 and t == 1))
              for oct in range(2):
                for half in range(2):
                    sl = slice(half * 576, (half + 1) * 576)
                    if oct == 0 or not use_dve:
                        nc.scalar.activation(
                            u_sb[oct][:, sl], pg[oct][:, sl],
                            mybir.ActivationFunctionType.Identity,
                            bias=pb_sb[:, oct:oct + 1])
                    else:
                        nc.vector.tensor_scalar(
                            u_sb[1][:, sl], pg[1][:, sl],
                            pb_sb[:, 1:2], None,
                            mybir.AluOpType.add, mybir.AluOpType.bypass)

        if mode != "n384":
            uov = uo.rearrange("(t p) n -> t p n", p=P)
            for oct in range(2):
                nc.sync.dma_start(uov[oct], u_sb[oct][:])
    nc.finalize()
    return nc


def _host_prep(x, conv1_w, conv1_b, prim_w, prim_b):
    """im2col + weight repack, fp16 cast. Returns per-core input maps."""
    sw = np.lib.stride_tricks.sliding_window_view(x, (9, 9), axis=(2, 3))
    # sw: [B,3,20,20,9,9] -> (c,ky,kx, oy,ox, b)
    pats = np.ascontiguousarray(sw.transpose(1, 4, 5, 2, 3, 0))
    p1_all = np.zeros((256, 400, B), np.float16)
    p1_all[:243] = pats.reshape(243, 400, B)
    w1t = np.zeros((256, 256), np.float16)
    w1t[:243] = conv1_w.reshape(256, 243).T
    w2t = np.ascontiguousarray(
        prim_w.reshape(256, 256, 9, 9).transpose(2, 3, 1, 0)
    ).reshape(81 * 256, 256).astype(np.float16)
    b1 = conv1_b.reshape(256, 1).astype(np.float32)
    pb = prim_b.reshape(256, 1).astype(np.float32)

    p1_all = p1_all.reshape(256, 400, NCORES, BC)
    in_maps = [{
        "p1": np.ascontiguousarray(
            p1_all[:, :, i, :]).reshape(256, FN),
        "w1": w1t, "w2": w2t, "b1": b1, "pb": pb,
    } for i in range(NCORES)]
    return in_maps


def _device_u(x, conv1_w, conv1_b, prim_w, prim_b, trace=False):
    """Run conv1+conv2 on 8 cores; return u [B, 256, 36] (pre-squash), results."""
    from concourse.bass_utils import run_bass_kernel_spmd

    in_maps = _host_prep(x, conv1_w, conv1_b, prim_w, prim_b)
    nc = _build_program()
    res = run_bass_kernel_spmd(nc, in_maps, core_ids=list(range(NCORES)),
                               trace=trace)
    u = np.concatenate([_unshard_u(r["u_out"]) for r in res.results], axis=0)
    return u, res


def _unshard_u(uo, mode=None):
    """Per-core u_out [256, 1152] -> [BC, 256, 36] with pos = y*6+x."""
    mode = mode or MODE
    if mode == "win288":   # cols (c4, y6, x6, b8)
        a = uo.reshape(256, 4, 36, 8).transpose(1, 3, 0, 2)
    elif mode == "n384":   # cols (g3, y6, xl2, b32); x = g*2 + xl
        return uo.reshape(256, 3, 6, 2, 32).transpose(
            4, 0, 2, 1, 3).reshape(BC, 256, 36)
    elif mode == "im2col":  # cols (y6, x6, b32)
        a = uo.reshape(256, 36, 32).transpose(2, 0, 1)
        return a.reshape(BC, 256, 36)
    else:
        raise ValueError(mode)
    return a.reshape(BC, 256, 36)


def _routing_host(u_c36, W):
    u = u_c36.reshape(B, NUM_ROUTES, NUM_PRIMARY).astype(np.float32)
    sq = np.sum(u * u, axis=-1, keepdims=True)
    u = sq * u / ((1.0 + sq) * np.sqrt(sq))
    # u_hat[b,r,m] (m = k*16+o): batched matmul over routes
    W2 = W.reshape(NUM_ROUTES, NUM_SHAPE * 16, NUM_PRIMARY).astype(np.float32)
    ut = np.ascontiguousarray(u.transpose(1, 2, 0))          # [1152, 8, B]
    uh = np.matmul(W2, ut)                                    # [1152, 160, B]
    uh4 = uh.reshape(NUM_ROUTES, NUM_SHAPE, 16, B)
    b_ij = np.zeros((NUM_ROUTES, NUM_SHAPE), np.float32)
    v = None
    for it in range(3):
        e = np.exp(b_ij - b_ij.max(axis=0, keepdims=True))
        c = e / e.sum(axis=0, keepdims=True)                  # [1152,10]
        s = np.einsum('rk,rkob->kob', c, uh4, optimize=True)  # [10,16,B]
        v = s * np.abs(s) / (1.0 + s * s)
        if it < 2:
            a = np.einsum('rkob,kob->rk', uh4, v, optimize=True) / B
            b_ij = b_ij + a
    return np.ascontiguousarray(v.transpose(2, 0, 1)).astype(np.float32)  # [B,10,16]


def _reference_numpy(x, conv1_w, conv1_b, prim_w, prim_b, W):
    """Pure-numpy fallback (also the oracle for the device conv path)."""
    sw = np.lib.stride_tricks.sliding_window_view(x, (9, 9), axis=(2, 3))
    pats = sw.transpose(0, 2, 3, 1, 4, 5).reshape(B * 400, 243)
    h = pats @ conv1_w.reshape(256, 243).T + conv1_b
    h = np.maximum(h, 0.0).reshape(B, 20, 20, 256)
    sw2 = np.lib.stride_tricks.sliding_window_view(h, (9, 9), axis=(1, 2))
    sw2 = sw2[:, ::2, ::2]                    # [B,6,6,256,9,9]
    pats2 = sw2.transpose(0, 1, 2, 4, 5, 3).reshape(B * 36, 81 * 256)
    w2t = prim_w.reshape(256, 256, 9, 9).transpose(2, 3, 1, 0).reshape(81 * 256, 256)
    u = pats2 @ w2t + prim_b.reshape(256)     # [B*36, 256]
    u = u.reshape(B, 36, 256).transpose(0, 2, 1).reshape(B, 256 * 36)
    return _routing_host(u, W)


def kernel(x, conv1_w, conv1_b, prim_w, prim_b, W):
    x = np.asarray(x, np.float32)
    conv1_w = np.asarray(conv1_w, np.float32)
    conv1_b = np.asarray(conv1_b, np.float32)
    prim_w = np.asarray(prim_w, np.float32)
    prim_b = np.asarray(prim_b, np.float32)
    W = np.asarray(W, np.float32)
    try:
        u, _ = _device_u(x, conv1_w, conv1_b, prim_w, prim_b)
        return _routing_host(u.reshape(B, 256 * 36), W)
    except Exception:
        import traceback
        traceback.print_exc()
        return _reference_numpy(x, conv1_w, conv1_b, prim_w, prim_b, W)



# revision 13
# speedup vs baseline: 1.9660x; 1.9660x over previous
"""CapsNet forward on 8 TRN2 NeuronCores — data-parallel over batch.

Device (per core, batch shard of 32): conv1 (9x9 s1 + relu) and the primary-caps
conv (9x9 s2) as fp16 matmuls (fp32 PSUM accumulate) against an SBUF-resident
feature map; conv1 is fed host-side im2col patches.  Host: squash + capsule
transform + 3 routing iterations (batch-global, tiny FLOP count) in numpy.

Layouts (per core):
  ps1 [256, 13058] fp16  rows = (t p) conv1 K-index (243 pad 256);
                         cols = [b1 | pb | w1 m 0..255 | p1 (y,x,b) 12800]
                         (biases ride as fp16 columns; one DMA chain feeds
                         bias+w1+first p1 chunk so the PE starts ~3.5 us in)
  w2  [128, 81*512] fp16 row p = k-major [k][t][m]: per-partition contiguous
                         1 KB per (k,t), DMA'd in groups of 3 k-tiles
  uo  [256, 1152]  f16   rows = (oct p) out-channel, cols (g3, y6, xl2, b32)

Structure (per core):
  conv1: chunks [256,512,768,1024*11] of the p1 stream; single rotating PSUM
         tag (bufs=3, 6 banks) so conv2's 2 banks are free before conv1 ends.
         Relu+bias evacuation alternates ACT (oct0) / DVE (oct1) into h_sb.
  conv2: 6 full-K streams, one per (oct, g); each = 162 MMs of N=384 into one
         PSUM bank, then DVE bias-add + store DMA overlapped with the next
         stream.  w2 groups queue on the SP ring directly after the p1
         chunks — no cross-ring deps; HBM stays saturated for p1 during
         conv1 and w2 streams in during conv1's PE tail / early conv2.

Cost-model notes: per-DMA fixed latency (SEQ 565 + HWDGE 625 serial + DGE
650 + sem 900 ns) dominates small transfers — hence one merged stream and
few, large DMAs.  PE is busy ~177 us of the ~186 us total; conv2's 972
N=384 fp16 matmuls are the roofline (fp8 DoubleRow fails the 2e-2 gate:
e4m3 quantization noise on a K=20736 contraction gives ~4e-2).
"""

import numpy as np

NUM_PRIMARY = 8
NUM_SHAPE = 10
NUM_ROUTES = 32 * 6 * 6  # 1152
B = 256
NCORES = 8
BC = B // NCORES  # 32
P = 128

HW1 = 19       # conv1 output grid actually read by conv2 (row/col 19 unused)
FN = BC * HW1 * HW1  # 11552 conv1 output cols per core, (y19, x19, b32)
HDR = 2 + 256  # bias cols + w1 cols prepended to the p1 stream
CHUNKS = [256, 512, 768] + [1024] * 9 + [800]  # p1 cols per chunk
KG = 3          # w2 k-tiles per DMA group
NKG = 27        # 81 / KG

MODE = "n384"   # output col layout (see _unshard_u)


def _build_program(use_dve=True, reps=1, mode=None):
    import concourse.mybir as mybir
    import concourse.tile as tile
    from concourse import bacc
    from contextlib import ExitStack

    f32 = mybir.dt.float32
    f16 = mybir.dt.float16
    nc = bacc.Bacc("TRN2", target_bir_lowering=False, debug=False,
                   num_devices=NCORES)
    ps1 = nc.dram_tensor("ps1", [256, HDR + FN], f16,
                         kind="ExternalInput").ap()
    w2 = nc.dram_tensor("w2", [P, 81 * 512], f16, kind="ExternalInput").ap()
    uo = nc.dram_tensor("u_out", [256, BC * 36], f16,
                        kind="ExternalOutput").ap()

    with tile.TileContext(nc) as tc, ExitStack() as ctx:
        ps1v = ps1.rearrange("(t p) n -> p t n", p=P)
        const = ctx.enter_context(tc.tile_pool(name="const", bufs=1))
        # header chunk: biases + w1 + first 256 p1 cols in one DMA
        c0 = const.tile([P, 2, HDR + CHUNKS[0]], f16)
        nc.sync.dma_start(c0[:], ps1v[:, :, :HDR + CHUNKS[0]])
        w1_sb = c0[:, :, 2:HDR]  # [:, t, m]
        # engines need f32 scalar operands: upcast the f16 bias cols once
        bias = const.tile([P, 2, 2], f32)  # [:, oct, 0]=b1, [:, oct, 1]=pb
        nc.vector.tensor_copy(bias[:], c0[:, :, 0:2])

        # PE warmup: the HAM clock gate holds the PE at 1.2 GHz until it has
        # seen ~3.4 us of sustained activity.  The first real matmul can't
        # start before the c0 DMA chain lands (~3.5 us), so burn that window
        # on dummy matmuls over a zeroed tile — conv1 then runs at 2.4 GHz
        # from its first instruction.
        wpool = ctx.enter_context(tc.tile_pool(name="warm", bufs=1))
        wt = wpool.tile([P, 128], f16)
        nc.vector.memset(wt[:], 0)
        with tc.tile_pool(name="warmps", bufs=1, space="PSUM") as warmps:
            wps = warmps.tile([P, 128], f32)
            for _ in range(34):
                nc.tensor.matmul(wps[:], wt[:], wt[:], start=True, stop=True)

        w2v = w2.rearrange("p (gk kg t m) -> p gk kg t m", gk=NKG, kg=KG, t=2)
        w2pool = ctx.enter_context(tc.tile_pool(name="w2", bufs=1))
        w2_sb = [w2pool.tile([P, KG, 2, 256], f16, tag=f"w2g_{gk}",
                             name=f"w2g_{gk}") for gk in range(NKG)]

        hpool = ctx.enter_context(tc.tile_pool(name="h", bufs=1))
        h_sb = [hpool.tile([P, FN], f16, tag=f"h{t}", name=f"h{t}")
                for t in range(2)]
        upool = ctx.enter_context(tc.tile_pool(name="u", bufs=2))

        # ---- conv1: h[m, pos] = relu(w1[:,m]^T @ p1[:,pos] + b1[m]) ----
        for rep in range(reps):
          # both PSUM pools coexist (3*2 + 2*1 = 8 banks): conv2's banks are
          # never owned by conv1 tiles, so its first matmul has no dep on
          # conv1's last evacuations
          with tc.tile_pool(name=f"p1pool{rep}", bufs=1) as p1pool, \
             tc.tile_pool(name=f"psum1_{rep}", bufs=3, space="PSUM") as psum1, \
             tc.tile_pool(name=f"psum2_{rep}", bufs=2, space="PSUM") as psum2:
            off = 0
            for ci, cw in enumerate(CHUNKS):
                if ci == 0:
                    pt = c0[:, :, HDR:]
                else:
                    # every chunk gets its own resident tile: the whole p1
                    # stream DMAs back-to-back with zero buffer-reuse waits,
                    # so the SP queue reaches the w2 stream ~10 us before
                    # conv1's PE work ends
                    ptt = p1pool.tile([P, 2, cw], f16, tag=f"pt{ci}",
                                      name=f"pt{rep}_{ci}")
                    pt = ptt[:]
                    nc.sync.dma_start(
                        pt, ps1v[:, :, HDR + off:HDR + off + cw])
                for oct in range(2):
                    ps = psum1.tile([P, 1024], f32, tag="ps",
                                    name=f"ps{rep}_{ci}_{oct}")
                    for n0 in range(0, cw, 512):
                        nn = min(512, cw - n0)
                        for t in range(2):
                            nc.tensor.matmul(
                                ps[:, n0:n0 + nn],
                                w1_sb[:, t, oct * P:(oct + 1) * P],
                                pt[:, t, n0:n0 + nn],
                                start=(t == 0), stop=(t == 1))
                    if oct == 0 or not use_dve:
                        nc.scalar.activation(
                            h_sb[oct][:, off:off + cw], ps[:, :cw],
                            mybir.ActivationFunctionType.Relu,
                            bias=bias[:, 0, 0:1])
                    else:
                        nc.vector.tensor_scalar(
                            h_sb[1][:, off:off + cw], ps[:, :cw],
                            bias[:, 1, 0:1], 0.0,
                            mybir.AluOpType.add, mybir.AluOpType.max)
                off += cw

            if rep == 0:
                # w2 stream: queued on the SP ring after the last p1 chunk,
                # so it starts exactly when p1 stops needing HBM.
                for gk in range(NKG):
                    nc.sync.dma_start(w2_sb[gk][:], w2v[:, gk])

            # ---- conv2: 6 full-K streams, one per (oct, g) ----
            hv = [h_sb[t][:].rearrange("p (y x b) -> p y x b", y=HW1, x=HW1)
                  for t in range(2)]
            uov_ = uo.rearrange("(o p) n -> o p n", p=P)
            for oct in range(2):
              for g in range(3):
                ps = psum2.tile([P, 384], f32, tag="pg",
                                name=f"pg{rep}_{oct}_{g}")
                for k in range(81):
                    ky, kx = divmod(k, 9)
                    for t in range(2):
                        nc.tensor.matmul(
                            ps[:],
                            w2_sb[k // KG][:, k % KG, t, oct * P:(oct + 1) * P],
                            hv[t][:, ky:ky + 11:2,
                                  kx + 4 * g:kx + 4 * g + 3:2, :],
                            start=(k == 0 and t == 0),
                            stop=(k == 80 and t == 1))
                ug = upool.tile([P, 384], f16, tag="ug",
                                name=f"ug{rep}_{oct}_{g}")
                nc.vector.tensor_scalar(
                    ug[:], ps[:], bias[:, oct, 1:2], None,
                    mybir.AluOpType.add, mybir.AluOpType.bypass)
                nc.sync.dma_start(uov_[oct][:, g * 384:(g + 1) * 384], ug[:])
    nc.finalize()
    return nc


def _host_prep(x, conv1_w, conv1_b, prim_w, prim_b):
    """im2col + weight repack, fp16 cast. Returns per-core input maps."""
    sw = np.lib.stride_tricks.sliding_window_view(x, (9, 9), axis=(2, 3))
    # sw: [B,3,20,20,9,9] -> (c,ky,kx, oy,ox, b)
    pats = np.ascontiguousarray(sw.transpose(1, 4, 5, 2, 3, 0))
    # conv2 (9x9 stride 2 on 20x20) never reads row/col 19: drop them
    p1_all = np.zeros((256, HW1 * HW1, B), np.float16)
    p1_all[:243] = pats.reshape(243, 20, 20, B)[:, :HW1, :HW1].reshape(
        243, HW1 * HW1, B)
    # header: [b1 | pb | w1 m0..255] per (t p) row
    w1t = np.zeros((256, 256), np.float32)
    w1t[:243] = conv1_w.reshape(256, 243).T
    hdr = np.zeros((256, HDR), np.float16)
    hdr[:, 0] = conv1_b.astype(np.float16)       # row (t p) = channel t*128+p
    hdr[:, 1] = prim_b.reshape(256).astype(np.float16)
    hdr[:, 2:] = w1t.astype(np.float16)
    # w2 [128, 81*512]: row p = k-major [k][t][m]
    w2t = np.ascontiguousarray(
        prim_w.reshape(256, 256, 9, 9).transpose(2, 3, 1, 0)
    ).reshape(81, 2, P, 256)
    w2n = np.ascontiguousarray(
        w2t.transpose(2, 0, 1, 3)).reshape(P, 81 * 512).astype(np.float16)

    p1_all = p1_all.reshape(256, HW1 * HW1, NCORES, BC)
    in_maps = [{
        "ps1": np.ascontiguousarray(np.concatenate(
            [hdr, p1_all[:, :, i, :].reshape(256, FN)], axis=1)),
        "w2": w2n,
    } for i in range(NCORES)]
    return in_maps


def _device_u(x, conv1_w, conv1_b, prim_w, prim_b, trace=False):
    """Run conv1+conv2 on 8 cores; return u [B, 256, 36] (pre-squash), results."""
    from concourse.bass_utils import run_bass_kernel_spmd

    in_maps = _host_prep(x, conv1_w, conv1_b, prim_w, prim_b)
    nc = _build_program()
    res = run_bass_kernel_spmd(nc, in_maps, core_ids=list(range(NCORES)),
                               trace=trace)
    u = np.concatenate([_unshard_u(r["u_out"]) for r in res.results], axis=0)
    return u, res


def _unshard_u(uo, mode=None):
    """Per-core u_out [256, 1152] -> [BC, 256, 36] with pos = y*6+x."""
    # cols (g3, y6, xl2, b32); x = g*2 + xl
    return np.asarray(uo).reshape(256, 3, 6, 2, 32).transpose(
        4, 0, 2, 1, 3).reshape(BC, 256, 36)


def _routing_host(u_c36, W):
    u = u_c36.reshape(B, NUM_ROUTES, NUM_PRIMARY).astype(np.float32)
    sq = np.sum(u * u, axis=-1, keepdims=True)
    u = sq * u / ((1.0 + sq) * np.sqrt(sq))
    # u_hat[b,r,m] (m = k*16+o): batched matmul over routes
    W2 = W.reshape(NUM_ROUTES, NUM_SHAPE * 16, NUM_PRIMARY).astype(np.float32)
    ut = np.ascontiguousarray(u.transpose(1, 2, 0))          # [1152, 8, B]
    uh = np.matmul(W2, ut)                                    # [1152, 160, B]
    uh4 = uh.reshape(NUM_ROUTES, NUM_SHAPE, 16, B)
    b_ij = np.zeros((NUM_ROUTES, NUM_SHAPE), np.float32)
    v = None
    for it in range(3):
        e = np.exp(b_ij - b_ij.max(axis=0, keepdims=True))
        c = e / e.sum(axis=0, keepdims=True)                  # [1152,10]
        s = np.einsum('rk,rkob->kob', c, uh4, optimize=True)  # [10,16,B]
        v = s * np.abs(s) / (1.0 + s * s)
        if it < 2:
            a = np.einsum('rkob,kob->rk', uh4, v, optimize=True) / B
            b_ij = b_ij + a
    return np.ascontiguousarray(v.transpose(2, 0, 1)).astype(np.float32)  # [B,10,16]


def _reference_numpy(x, conv1_w, conv1_b, prim_w, prim_b, W):
    """Pure-numpy fallback (also the oracle for the device conv path)."""
    sw = np.lib.stride_tricks.sliding_window_view(x, (9, 9), axis=(2, 3))
    pats = sw.transpose(0, 2, 3, 1, 4, 5).reshape(B * 400, 243)
    h = pats @ conv1_w.reshape(256, 243).T + conv1_b
    h = np.maximum(h, 0.0).reshape(B, 20, 20, 256)
    sw2 = np.lib.stride_tricks.sliding_window_view(h, (9, 9), axis=(1, 2))
    sw2 = sw2[:, ::2, ::2]                    # [B,6,6,256,9,9]
    pats2 = sw2.transpose(0, 1, 2, 4, 5, 3).reshape(B * 36, 81 * 256)
    w2t = prim_w.reshape(256, 256, 9, 9).transpose(2, 3, 1, 0).reshape(81 * 256, 256)
    u = pats2 @ w2t + prim_b.reshape(256)     # [B*36, 256]
    u = u.reshape(B, 36, 256).transpose(0, 2, 1).reshape(B, 256 * 36)
    return _routing_host(u, W)


def kernel(x, conv1_w, conv1_b, prim_w, prim_b, W):
    x = np.asarray(x, np.float32)
    conv1_w = np.asarray(conv1_w, np.float32)
    conv1_b = np.asarray(conv1_b, np.float32)
    prim_w = np.asarray(prim_w, np.float32)
    prim_b = np.asarray(prim_b, np.float32)
    W = np.asarray(W, np.float32)
    try:
        u, _ = _device_u(x, conv1_w, conv1_b, prim_w, prim_b)
        return _routing_host(u.reshape(B, 256 * 36), W)
    except Exception:
        import traceback
        traceback.print_exc()
        return _reference_numpy(x, conv1_w, conv1_b, prim_w, prim_b, W)
